# revision 6
# baseline (speedup 1.0000x reference)
"""Trainium2 Bass kernel for nn_AttentionHead (B=4, N=2048, d_model=1024, d_k=64).

Sharding: data-parallel over (batch, query-half) -> 8 cores. Each core gets
q^T[b, :, h*1024:(h+1)*1024], full k^T[b], v^T[b] (host pre-transposes so
d_model lands on SBUF partitions; projections contract d_model), plus the
replicated projection weights.

Per-core device program (all matmuls bf16 with fp32 PSUM accumulation):
  1. q_^T = Wq^T q^T + bq   [64, 1024]   (8 K-tiles of d_model accumulated)
     k_^T = Wk^T k^T + bk   [64, 2048]
     v_^T = Wv^T v^T + bv   [64, 2048] (fp32), PE-transposed into
     v_aug [2048, 65] with a ones column appended.
  2. scores^T tile = k_^T_tile.T @ q_^T (per 128-wide k-tile), exp via ACT
     (scale=1/8 folded in), then out_aug^T[65, 1024] += v_aug_tile.T @ e_tile.
     The ones column makes row 64 the softmax denominator, so no separate
     reduction is needed (exp-softmax without max-subtraction, faithful to
     the reference).
  3. PE-transpose out_aug^T back to [1024, 65], divide by the denominator
     column in fp32, DMA out.
"""

import numpy as np
import ml_dtypes

import concourse.bass as bass
import concourse.tile as tile
from concourse import mybir
from concourse.bass_utils import run_bass_kernel_spmd
from concourse.masks import make_identity

B, N, DM, DK = 4, 2048, 1024, 64
NCORES = 8
NQ = N // 2          # queries per core
NK = N               # keys per core
P = 128
NDM = DM // P        # 8 d_model tiles
NKT = NK // P        # 16 key tiles
NQC = 512            # query chunk (one PSUM bank)
NQCH = NQ // NQC     # 2 query chunks
DT = mybir.dt.bfloat16
F32 = mybir.dt.float32
BF = ml_dtypes.bfloat16


# --- walrus wait legalization -------------------------------------------------
# The walrus build in this container accepts at most 1 sync wait + 1 sync
# update per instruction (2 for EventSemaphore). Excess WAITS are hoisted
# onto same-engine NoOps placed just before (queues issue in order, so the
# gating is preserved). Updates are completion-signals and stay put.

def _caps(inst):
    opcode = type(inst).__name__
    if opcode == "InstEventSemaphore":
        return 2, 2
    return 1, 1


def _legalize_waits(nc):
    for f in nc.m.functions:
        for bb in f.blocks:
            out = []
            changed = False
            for inst in bb.instructions:
                si = inst.sync_info
                waits = list(si.on_wait) if si is not None else []
                updates = list(si.on_update) if si is not None else []
                wcap, ucap = _caps(inst)
                if len(waits) <= wcap and len(updates) <= ucap:
                    out.append(inst)
                    continue
                changed = True
                keep_w = waits[len(waits) - wcap:] if wcap else []
                extra_w = waits[: len(waits) - wcap] if wcap else waits
                # Updates signal instruction COMPLETION (writes landed);
                # a following NoOp fires at issue time instead, which races
                # consumers against in-flight writes. Never hoist them.
                assert len(updates) <= ucap, (
                    f"{inst.name}: {len(updates)} sync updates exceed the "
                    f"per-instruction cap and cannot be hoisted safely"
                )
                keep_u = updates
                extra_u = []
                for w in extra_w:
                    nop = mybir.InstNoOp(
                        name=nc.get_next_instruction_name(), ins=[], outs=[]
                    )
                    nop.engine = inst.engine
                    nop.sync_info = mybir.SyncInfo(on_wait=[w], on_update=[])
                    out.append(nop)
                inst.sync_info = mybir.SyncInfo(on_wait=keep_w, on_update=keep_u)
                out.append(inst)
                for u in extra_u:
                    nop = mybir.InstNoOp(
                        name=nc.get_next_instruction_name(), ins=[], outs=[]
                    )
                    nop.engine = inst.engine
                    nop.sync_info = mybir.SyncInfo(on_wait=[], on_update=[u])
                    out.append(nop)
            if changed:
                bb.instructions = out


# --- device program -----------------------------------------------------------

def _build(reps=1):
    nc = bass.Bass()
    qT_d = nc.dram_tensor("qT", [DM, NQ], DT, kind="ExternalInput")
    kT_d = nc.dram_tensor("kT", [DM, NK], DT, kind="ExternalInput")
    vT_d = nc.dram_tensor("vT", [DM, NK], DT, kind="ExternalInput")
    wq_d = nc.dram_tensor("wq", [DM, DK], DT, kind="ExternalInput")
    wk_d = nc.dram_tensor("wk", [DM, DK], DT, kind="ExternalInput")
    wv_d = nc.dram_tensor("wv", [DM, DK], DT, kind="ExternalInput")
    bq_d = nc.dram_tensor("bq", [DK, 1], F32, kind="ExternalInput")
    bk_d = nc.dram_tensor("bk", [DK, 1], F32, kind="ExternalInput")
    bv_d = nc.dram_tensor("bv", [DK, 1], F32, kind="ExternalInput")
    out_d = nc.dram_tensor("out", [NQ, DK], F32, kind="ExternalOutput")

    with tile.TileContext(nc) as tc:
      for _rep in range(reps):
        with tc.tile_pool(name="persist", bufs=1) as persist:
            wq_sb = persist.tile([P, NDM, DK], DT, tag="wq_sb")
            wk_sb = persist.tile([P, NDM, DK], DT, tag="wk_sb")
            wv_sb = persist.tile([P, NDM, DK], DT, tag="wv_sb")
            nc.sync.dma_start(wq_sb[:], wq_d.rearrange("(o p) k -> p o k", p=P))
            nc.sync.dma_start(wk_sb[:], wk_d.rearrange("(o p) k -> p o k", p=P))
            nc.sync.dma_start(wv_sb[:], wv_d.rearrange("(o p) k -> p o k", p=P))
            bq_sb = persist.tile([DK, 1], F32, tag="bq_sb")
            bk_sb = persist.tile([DK, 1], F32, tag="bk_sb")
            bv_sb = persist.tile([DK, 1], F32, tag="bv_sb")
            nc.sync.dma_start(bq_sb[:], bq_d[:])
            nc.sync.dma_start(bk_sb[:], bk_d[:])
            nc.sync.dma_start(bv_sb[:], bv_d[:])
            ident = persist.tile([P, P], F32, tag="ident")
            make_identity(nc, ident[:])

            # projected activations; k/q padded to 128 partitions with zeros
            # so the scores matmul can contract over a full 128 rows.
            k_sbT = persist.tile([P, NK], DT, tag="k_sbT")
            q_sbT = persist.tile([P, NQ], DT, tag="q_sbT")
            v_sbT = persist.tile([P, NK], F32, tag="v_sbT")
            v_aug = persist.tile([P, NKT, DK + 1], DT, tag="v_aug")
            out_sb = persist.tile([P, NQ // P, DK], F32, tag="out_sb")
            nc.vector.memset(k_sbT[:], 0.0)
            nc.vector.memset(q_sbT[:], 0.0)
            nc.vector.memset(v_sbT[:], 0.0)
            nc.gpsimd.memset(v_aug[:], 1.0)  # ones column survives at [:, :, 64]

            # ---- projections: x_^T[64, n] += Wx_tile.T @ x^T_tile ----
            with (
                tc.tile_pool(name="xt", bufs=3) as xtp,
                tc.tile_pool(name="psproj", bufs=1, space="PSUM") as psp,
            ):
                def project(x_d, w_sb, n, writeback):
                    nch = n // NQC
                    ps = [
                        psp.tile([DK, NQC], F32, tag=f"psproj{j}", name=f"psproj{j}")
                        for j in range(nch)
                    ]
                    for dmt in range(NDM):
                        xt = xtp.tile([P, n], DT, tag="xt")
                        nc.sync.dma_start(
                            xt[:], x_d[dmt * P:(dmt + 1) * P, :]
                        )
                        for j in range(nch):
                            nc.tensor.matmul(
                                ps[j][:],
                                w_sb[:, dmt, :],
                                xt[:, j * NQC:(j + 1) * NQC],
                                start=(dmt == 0),
                                stop=(dmt == NDM - 1),
                            )
                    for j in range(nch):
                        writeback(j, ps[j])

                project(
                    kT_d, wk_sb, NK,
                    lambda j, ps: nc.vector.tensor_scalar_add(
                        k_sbT[0:DK, j * NQC:(j + 1) * NQC], ps[:], bk_sb[:]
                    ),
                )
                project(
                    qT_d, wq_sb, NQ,
                    lambda j, ps: nc.vector.tensor_scalar_add(
                        q_sbT[0:DK, j * NQC:(j + 1) * NQC], ps[:], bq_sb[:]
                    ),
                )
                project(
                    vT_d, wv_sb, NK,
                    lambda j, ps: nc.vector.tensor_scalar_add(
                        v_sbT[0:DK, j * NQC:(j + 1) * NQC], ps[:], bv_sb[:]
                    ),
                )

            # ---- v_^T[64, NK] -> v_aug[nk, 65] via exact fp32 PE transpose ----
            with tc.tile_pool(name="pst", bufs=2, space="PSUM") as pst:
                for t in range(NKT):
                    pt = pst.tile([P, P], F32, tag="pst")
                    nc.tensor.transpose(
                        pt[:], v_sbT[:, t * P:(t + 1) * P], ident[:]
                    )
                    nc.vector.tensor_copy(v_aug[:, t, 0:DK], pt[:, 0:DK])

            # ---- attention ----
            with tc.tile_pool(name="psout", bufs=1, space="PSUM") as pso:
                oacc = [
                    pso.tile([DK + 1, NQC], F32, tag=f"oacc{j}", name=f"oacc{j}")
                    for j in range(NQCH)
                ]
                with (
                    tc.tile_pool(name="epool", bufs=3) as ep,
                    tc.tile_pool(name="psscore", bufs=2, space="PSUM") as pss,
                ):
                    for t in range(NKT):
                        s = pss.tile([P, NQ], F32, tag="psscore")
                        for j in range(NQCH):
                            nc.tensor.matmul(
                                s[:, j * NQC:(j + 1) * NQC],
                                k_sbT[:, t * P:(t + 1) * P],
                                q_sbT[:, j * NQC:(j + 1) * NQC],
                                start=True,
                                stop=True,
                            )
                        e = ep.tile([P, NQ], DT, tag="e")
                        nc.scalar.activation(
                            e[:], s[:],
                            mybir.ActivationFunctionType.Exp,
                            scale=1.0 / np.sqrt(np.float32(DK)),
                        )
                        for j in range(NQCH):
                            nc.tensor.matmul(
                                oacc[j][:],
                                v_aug[:, t, :],
                                e[:, j * NQC:(j + 1) * NQC],
                                start=(t == 0),
                                stop=(t == NKT - 1),
                            )

                # ---- normalize + transpose back ----
                with (
                    tc.tile_pool(name="fin", bufs=2) as fin,
                    tc.tile_pool(name="psfin", bufs=2, space="PSUM") as psf,
                ):
                    for t in range(NQ // P):
                        j, c = divmod(t, NQC // P)
                        osb = fin.tile([P, P], F32, tag="osb")
                        nc.vector.memset(osb[DK:P, :], 0.0)
                        nc.vector.tensor_copy(
                            osb[0:DK + 1, :], oacc[j][:, c * P:(c + 1) * P]
                        )
                        pt = psf.tile([P, P], F32, tag="psfin")
                        nc.tensor.transpose(pt[:], osb[:], ident[:])
                        rc = fin.tile([P, 1], F32, tag="rc")
                        nc.vector.reciprocal(rc[:], pt[:, DK:DK + 1])
                        nc.vector.tensor_scalar_mul(
                            out_sb[:, t, :], pt[:, 0:DK], rc[:]
                        )
                    nc.sync.dma_start(
                        out_d.rearrange("(o p) k -> p o k", p=P), out_sb[:]
                    )

    _legalize_waits(nc)
    return nc


_nc_cache = None


def _get_nc():
    global _nc_cache
    if _nc_cache is None:
        _nc_cache = _build()
    return _nc_cache


def _marshal(q, k, v, Wq, bq, Wk, bk, Wv, bv):
    """Host-side layout prep: transpose to [B, d_model, N], cast to bf16,
    shard over (batch, query-half)."""
    qT = np.ascontiguousarray(np.transpose(np.asarray(q), (0, 2, 1))).astype(BF)
    kT = np.ascontiguousarray(np.transpose(np.asarray(k), (0, 2, 1))).astype(BF)
    vT = np.ascontiguousarray(np.transpose(np.asarray(v), (0, 2, 1))).astype(BF)
    wq = np.asarray(Wq).astype(BF)
    wk = np.asarray(Wk).astype(BF)
    wv = np.asarray(Wv).astype(BF)
    bqc = np.asarray(bq, dtype=np.float32).reshape(DK, 1)
    bkc = np.asarray(bk, dtype=np.float32).reshape(DK, 1)
    bvc = np.asarray(bv, dtype=np.float32).reshape(DK, 1)
    in_maps = []
    for c in range(NCORES):
        bi, h = divmod(c, 2)
        in_maps.append({
            "qT": np.ascontiguousarray(qT[bi][:, h * NQ:(h + 1) * NQ]),
            "kT": kT[bi],
            "vT": vT[bi],
            "wq": wq, "wk": wk, "wv": wv,
            "bq": bqc, "bk": bkc, "bv": bvc,
        })
    return in_maps


def _unmarshal(results):
    out = np.empty((B, N, DK), np.float32)
    for c in range(NCORES):
        bi, h = divmod(c, 2)
        out[bi, h * NQ:(h + 1) * NQ] = results[c]["out"]
    return out


def kernel(q, k, v, Wq, bq, Wk, bk, Wv, bv):
    in_maps = _marshal(q, k, v, Wq, bq, Wk, bk, Wv, bv)
    res = run_bass_kernel_spmd(_get_nc(), in_maps, core_ids=list(range(NCORES)))
    return _unmarshal(res.results)


# revision 38
# speedup vs baseline: 844.4899x; 844.4899x over previous
"""Trainium2 Bass kernel for nn_AttentionHead (B=4, N=2048, d_model=1024, d_k=64).

Sharding: data-parallel over (batch, query-half) -> 8 cores. Each core gets
q^T[b, :, h*1024:(h+1)*1024], full k^T[b], v^T[b] (host pre-transposes so
d_model lands on SBUF partitions; projections contract d_model), plus the
replicated projection weights.

Per-core device program (all matmuls bf16 with fp32 PSUM accumulation):
  1. q_^T = Wq^T q^T + bq   [64, 1024]   (8 K-tiles of d_model accumulated)
     k_^T = Wk^T k^T + bk   [64, 2048]
     v_^T = Wv^T v^T + bv   [64, 2048] (fp32), PE-transposed into
     v_aug [2048, 65] with a ones column appended.
  2. scores^T tile = k_^T_tile.T @ q_^T (per 128-wide k-tile), exp via ACT
     (scale=1/8 folded in), then out_aug^T[65, 1024] += v_aug_tile.T @ e_tile.
     The ones column makes row 64 the softmax denominator, so no separate
     reduction is needed (exp-softmax without max-subtraction, faithful to
     the reference).
  3. PE-transpose out_aug^T back to [1024, 65], divide by the denominator
     column in fp32, DMA out.
"""

import numpy as np
import ml_dtypes

import concourse.bass as bass
import concourse.tile as tile
from concourse import mybir
from concourse.bass_utils import run_bass_kernel_spmd
from concourse.masks import make_identity

B, N, DM, DK = 4, 2048, 1024, 64
NCORES = 8
NQ = N // 2          # queries per core
NK = N               # keys per core
P = 128
NDM = DM // P        # 8 d_model tiles
NKT = NK // P        # 16 key tiles
NQC = 512            # query chunk (one PSUM bank)
NQCH = NQ // NQC     # 2 query chunks
DT = mybir.dt.bfloat16
F32 = mybir.dt.float32
BF = ml_dtypes.bfloat16


# --- walrus wait legalization -------------------------------------------------
# The walrus build in this container accepts at most 1 sync wait + 1 sync
# update per instruction (2 for EventSemaphore). Excess WAITS are hoisted
# onto same-engine NoOps placed just before (queues issue in order, so the
# gating is preserved). Updates are completion-signals and stay put.

def _caps(inst):
    opcode = type(inst).__name__
    if opcode == "InstEventSemaphore":
        return 2, 2
    return 1, 1


def _legalize_waits(nc):
    for f in nc.m.functions:
        for bb in f.blocks:
            out = []
            changed = False
            for inst in bb.instructions:
                si = inst.sync_info
                waits = list(si.on_wait) if si is not None else []
                updates = list(si.on_update) if si is not None else []
                wcap, ucap = _caps(inst)
                if len(waits) <= wcap and len(updates) <= ucap:
                    out.append(inst)
                    continue
                changed = True
                keep_w = waits[len(waits) - wcap:] if wcap else []
                extra_w = waits[: len(waits) - wcap] if wcap else waits
                # Updates signal instruction COMPLETION (writes landed);
                # a following NoOp fires at issue time instead, which races
                # consumers against in-flight writes. Never hoist them.
                assert len(updates) <= ucap, (
                    f"{inst.name}: {len(updates)} sync updates exceed the "
                    f"per-instruction cap and cannot be hoisted safely"
                )
                keep_u = updates
                extra_u = []
                for w in extra_w:
                    nop = mybir.InstNoOp(
                        name=nc.get_next_instruction_name(), ins=[], outs=[]
                    )
                    nop.engine = inst.engine
                    nop.sync_info = mybir.SyncInfo(on_wait=[w], on_update=[])
                    out.append(nop)
                inst.sync_info = mybir.SyncInfo(on_wait=keep_w, on_update=keep_u)
                out.append(inst)
                for u in extra_u:
                    nop = mybir.InstNoOp(
                        name=nc.get_next_instruction_name(), ins=[], outs=[]
                    )
                    nop.engine = inst.engine
                    nop.sync_info = mybir.SyncInfo(on_wait=[], on_update=[u])
                    out.append(nop)
            if changed:
                bb.instructions = out


# --- device program -----------------------------------------------------------

def _build(reps=1):
    nc = bass.Bass()
    qT_d = nc.dram_tensor("qT", [DM, NQ], DT, kind="ExternalInput")
    kT_d = nc.dram_tensor("kT", [DM, NK], DT, kind="ExternalInput")
    vT_d = nc.dram_tensor("vT", [DM, NK], DT, kind="ExternalInput")
    w3_d = nc.dram_tensor("w3", [P, NDM * 3 * DK], DT, kind="ExternalInput")
    b3_d = nc.dram_tensor("b3", [DK, 3], F32, kind="ExternalInput")
    out_d = nc.dram_tensor("out", [DK, NQ], F32, kind="ExternalOutput")

    NCH_K = NK // NQC   # 4 key chunks
    NCH_Q = NQ // NQC   # 2 query chunks
    NTQ = NQ // P       # 8 output row-tiles
    EXP = mybir.ActivationFunctionType.Exp
    IDF = mybir.ActivationFunctionType.Identity
    SCALE = 1.0 / float(np.sqrt(np.float32(DK)))

    with tile.TileContext(nc) as tc:
      for _rep in range(reps):
        with tc.tile_pool(name="persist", bufs=1) as persist:
            w3_sb = persist.tile([P, NDM, 3 * DK], DT, tag="w3_sb")
            b3_sb = persist.tile([DK, 3], F32, tag="b3_sb")
            ident = persist.tile([P, P], F32, tag="ident")
            identb = persist.tile([P, P], DT, tag="identb")
            k_sbT = persist.tile([P, NK], DT, tag="k_sbT")
            q_sbT = persist.tile([P, NQ], DT, tag="q_sbT")
            v_sbT = persist.tile([P, NK], DT, tag="v_sbT")
            v_aug = persist.tile([P, NKT, DK + 1], DT, tag="v_aug")
            e_all = persist.tile([P, NKT, NQ], DT, tag="e_all")
            out_sbT = persist.tile([DK, NQ], F32, tag="out_sbT")
            onesr = persist.tile([1, DK], DT, tag="onesr")

            with (
                tc.tile_pool(name="xt", bufs=1) as xtp,
                tc.tile_pool(name="psout", bufs=1, space="PSUM") as pso,
            ):
                oacc = [
                    pso.tile([DK + 1, NQC], F32, tag=f"oacc{h}", name=f"oacc{h}")
                    for h in range(NCH_Q)
                ]
                # kq DMA stream: kt0, w3, qt0, kt1, qt1, kt2, kt3, b3
                kts, qts = [], []
                KT_GRAN = [(0, 3), (3, 3), (6, 1), (7, 1)]  # (dmt0, n_dmt)
                def dma_kt(i, eng):
                    d0, nd = KT_GRAN[i]
                    t_ = xtp.tile([P, nd, NK], DT, tag=f"kt{i}", name=f"kt{i}")
                    eng.dma_start(
                        t_[:], kT_d[d0 * P:(d0 + nd) * P, :].rearrange(
                            "(o p) n -> p o n", p=P))
                    kts.append(t_)
                def dma_qt(i, eng):
                    t_ = xtp.tile([P, 4, NQ], DT, tag=f"qt{i}", name=f"qt{i}")
                    eng.dma_start(
                        t_[:], qT_d[i * 4 * P:(i + 1) * 4 * P, :].rearrange(
                            "(o p) n -> p o n", p=P))
                    qts.append(t_)
                # alternate SP/ACT HWDGE queues so per-DMA setup overlaps the
                # serialized transfers (ACT is otherwise idle this early)
                dma_kt(0, nc.sync)
                nc.scalar.dma_start(
                    w3_sb[:], w3_d.rearrange("p (o k) -> p o k", o=NDM))
                nc.scalar.dma_start(b3_sb[:], b3_d[:])
                dma_qt(0, nc.scalar)
                dma_kt(1, nc.sync)
                dma_qt(1, nc.scalar)
                dma_kt(2, nc.sync)
                dma_kt(3, nc.scalar)

                make_identity(nc, ident[:])
                nc.vector.tensor_copy(identb[:], ident[:])
                # preload the exp table set while the DMA stream runs
                nc.scalar.activation(
                    e_all[0:1, 0, 0:1], ident[0:1, 0:1], EXP, scale=1.0)
                nc.gpsimd.memset(k_sbT[DK:P, :], 0.0)
                nc.gpsimd.memset(q_sbT[DK:P, :], 0.0)
                nc.gpsimd.memset(v_sbT[DK:P, :], 0.0)
                nc.gpsimd.memset(v_aug[:], 1.0)  # ones col at [:, :, 64]
                nc.gpsimd.memset(onesr[:], 1.0)

                # ---- k/q projections riding the DMA stream ----
                with tc.tile_pool(name="pskq", bufs=1, space="PSUM") as pskq:
                    psq = [pskq.tile([DK, NQC], F32, tag=f"psq{j}", name=f"psq{j}")
                           for j in range(NCH_Q)]
                    psk = [pskq.tile([DK, NQC], F32, tag=f"psk{j}", name=f"psk{j}")
                           for j in range(NCH_K)]

                    def kt_view(dmt):
                        for i, (d0, nd) in enumerate(KT_GRAN):
                            if d0 <= dmt < d0 + nd:
                                return kts[i][:, dmt - d0, :]
                        raise AssertionError(dmt)
                    def kp(dmts):
                        for dmt in dmts:
                            kv = kt_view(dmt)
                            for j in range(NCH_K):
                                nc.tensor.matmul(
                                    psk[j][:], w3_sb[:, dmt, DK:2 * DK],
                                    kv[:, j * NQC:(j + 1) * NQC],
                                    start=(dmt == 0), stop=(dmt == NDM - 1))
                    def qp(dmts):
                        for dmt in dmts:
                            for j in range(NCH_Q):
                                nc.tensor.matmul(
                                    psq[j][:], w3_sb[:, dmt, 0:DK],
                                    qts[dmt // 4][:, dmt % 4,
                                                  j * NQC:(j + 1) * NQC],
                                    start=(dmt == 0), stop=(dmt == NDM - 1))
                    kp([0])
                    kp([1, 2])
                    qp([0, 1, 2, 3])
                    kp([3, 4, 5])
                    # keep the PE warm while qt1/kt granules land; results
                    # are discarded (first real oacc matmul resets the bank)
                    for _w in range(14):
                        nc.tensor.matmul(
                            oacc[0][0:DK, 0:NQC], w3_sb[:, 0, 0:DK],
                            k_sbT[:, NK - NQC:NK], start=True, stop=True)
                    qp([4, 5, 6, 7])
                    kp([6])
                    kp([7])
                    # writebacks: k0,k1 on ACT; q0,q1,k2,k3 on DVE — so the
                    # psum banks reused by the scores pool free earliest
                    nc.scalar.activation(
                        k_sbT[0:DK, 0:NQC], psk[0][:], IDF, bias=b3_sb[:, 1:2])
                    nc.vector.tensor_scalar_add(
                        q_sbT[0:DK, 0:NQC], psq[0][:], b3_sb[:, 0:1])
                    nc.scalar.activation(
                        k_sbT[0:DK, NQC:2 * NQC], psk[1][:], IDF,
                        bias=b3_sb[:, 1:2])
                    nc.vector.tensor_scalar_add(
                        q_sbT[0:DK, NQC:2 * NQC], psq[1][:], b3_sb[:, 0:1])
                    nc.vector.tensor_scalar_add(
                        k_sbT[0:DK, 2 * NQC:3 * NQC], psk[2][:], b3_sb[:, 1:2])
                    nc.vector.tensor_scalar_add(
                        k_sbT[0:DK, 3 * NQC:4 * NQC], psk[3][:], b3_sb[:, 1:2])

                # ---- attention (t-major) with pipelined v chain ----
                # vT loaded chunk-major: granule j = all d_model for keys
                # [j*512, (j+1)*512); its projection, writeback, transposes
                # and the out-matmuls are interleaved into the scores/exp loop.
                vts = []
                for j in range(NCH_K):
                    vt = xtp.tile([P, NDM, NQC], DT, tag=f"vt{j}", name=f"vt{j}")
                    nc.sync.dma_start(
                        vt[:], vT_d[:, j * NQC:(j + 1) * NQC].rearrange(
                            "(o p) n -> p o n", p=P))
                    vts.append(vt)
                with (
                    tc.tile_pool(name="psscore", bufs=2, space="PSUM") as pss,
                    tc.tile_pool(name="psv", bufs=1, space="PSUM") as psvp,
                ):
                    psva = [None]

                    def v_mm(j, dmts):
                        if dmts[0] == 0:
                            psva[0] = psvp.tile(
                                [DK, NQC], F32, tag="psvacc", name=f"psva{j}")
                        for dmt in dmts:
                            nc.tensor.matmul(
                                psva[0][:], w3_sb[:, dmt, 2 * DK:3 * DK],
                                vts[j][:, dmt, :],
                                start=(dmt == 0), stop=(dmt == NDM - 1))
                    def v_wb(j):
                        nc.vector.tensor_scalar_add(
                            v_sbT[0:DK, j * NQC:(j + 1) * NQC], psva[0][:],
                            b3_sb[:, 2:3])
                    def v_tr(ts_):
                        for t_ in ts_:
                            pt = psvp.tile([P, P], DT, tag="psvb", name=f"pvb{t_}")
                            nc.tensor.transpose(
                                pt[:], v_sbT[:, t_ * P:(t_ + 1) * P], identb[:])
                            nc.vector.tensor_copy(v_aug[:, t_, 0:DK],
                                                  pt[:, 0:DK])
                    def o_mm(tp):
                        for h in range(NCH_Q):
                            nc.tensor.matmul(
                                oacc[h][:], v_aug[:, tp, :],
                                e_all[:, tp, h * NQC:(h + 1) * NQC],
                                start=(tp == 0), stop=(tp == NKT - 1))

                    # per-slot v-pipeline work: chunk j MMs at slots 4j+1/4j+2,
                    # writeback after, transposes at 4j+3/4j+4
                    vwork = {}
                    for j in range(NCH_K):
                        vwork.setdefault(2 * j + 1, []).append(
                            lambda j=j: v_mm(j, [0, 1, 2, 3]))
                        vwork.setdefault(2 * j + 2, []).append(
                            lambda j=j: (v_mm(j, [4, 5, 6, 7]), v_wb(j)))
                        vwork.setdefault(2 * j + 3, []).append(
                            lambda j=j: v_tr([4 * j, 4 * j + 1]))
                        vwork.setdefault(2 * j + 4, []).append(
                            lambda j=j: v_tr([4 * j + 2, 4 * j + 3]))

                    ODELAY = 5
                    for t in range(NKT):
                        sc = pss.tile([P, NQ], F32, tag="psscore")
                        for h in range(NCH_Q):
                            nc.tensor.matmul(
                                sc[:, h * NQC:(h + 1) * NQC],
                                k_sbT[:, t * P:(t + 1) * P],
                                q_sbT[:, h * NQC:(h + 1) * NQC],
                                start=True, stop=True)
                        nc.scalar.activation(
                            e_all[:, t, :], sc[:], EXP, scale=SCALE)
                        for fn in vwork.get(t, []):
                            fn()
                        if t >= ODELAY:
                            o_mm(t - ODELAY)
                    for fn in vwork.get(NKT, []):
                        fn()
                    for tp in range(NKT - ODELAY, NKT):
                        o_mm(tp)

                # ---- normalize in transposed layout + store ----
                # out^T[dk, nq] = oacc[0:64] * (1/oacc[64]) ; the reciprocal
                # row is broadcast across partitions with a K=1 matmul.
                with (
                    tc.tile_pool(name="fin", bufs=2) as fin,
                    tc.tile_pool(name="psfin", bufs=2, space="PSUM") as psf,
                ):
                    for h in range(NCH_Q):
                        rcr = fin.tile([1, NQC], F32, tag="rcr")
                        nc.vector.reciprocal(rcr[:], oacc[h][DK:DK + 1, :])
                        rcb = fin.tile([1, NQC], DT, tag="rcb")
                        nc.vector.tensor_copy(rcb[:], rcr[:])
                        pb = psf.tile([DK, NQC], F32, tag="psfin")
                        nc.tensor.matmul(
                            pb[:], onesr[:], rcb[:], start=True, stop=True)
                        rcf = fin.tile([DK, NQC], F32, tag="rcf")
                        nc.scalar.copy(rcf[:], pb[:])
                        nc.vector.tensor_tensor(
                            out_sbT[:, h * NQC:(h + 1) * NQC],
                            oacc[h][0:DK, :], rcf[:], mybir.AluOpType.mult)
                        (nc.sync if h == 0 else nc.scalar).dma_start(
                            out_d[:, h * NQC:(h + 1) * NQC],
                            out_sbT[:, h * NQC:(h + 1) * NQC])
    _legalize_waits(nc)
    return nc


_nc_cache = None


def _get_nc():
    global _nc_cache
    if _nc_cache is None:
        _nc_cache = _build()
    return _nc_cache


def _marshal(q, k, v, Wq, bq, Wk, bk, Wv, bv):
    """Host-side layout prep: transpose to [B, d_model, N], cast to bf16,
    shard over (batch, query-half)."""
    qT = np.ascontiguousarray(np.transpose(np.asarray(q), (0, 2, 1))).astype(BF)
    kT = np.ascontiguousarray(np.transpose(np.asarray(k), (0, 2, 1))).astype(BF)
    vT = np.ascontiguousarray(np.transpose(np.asarray(v), (0, 2, 1))).astype(BF)
    w3 = np.concatenate(
        [np.asarray(Wq), np.asarray(Wk), np.asarray(Wv)], axis=1
    ).astype(BF)
    # [1024, 192] -> [128, 8*192] partition-major so the DMA is contiguous
    w3 = np.ascontiguousarray(
        w3.reshape(NDM, P, 3 * DK).transpose(1, 0, 2).reshape(P, NDM * 3 * DK)
    )
    b3 = np.stack(
        [np.asarray(bq), np.asarray(bk), np.asarray(bv)], axis=1
    ).astype(np.float32)
    in_maps = []
    for c in range(NCORES):
        bi, h = divmod(c, 2)
        in_maps.append({
            "qT": np.ascontiguousarray(qT[bi][:, h * NQ:(h + 1) * NQ]),
            "kT": kT[bi],
            "vT": vT[bi],
            "w3": w3, "b3": b3,
        })
    return in_maps


def _unmarshal(results):
    out = np.empty((B, N, DK), np.float32)
    for c in range(NCORES):
        bi, h = divmod(c, 2)
        out[bi, h * NQ:(h + 1) * NQ] = results[c]["out"].T
    return out


def kernel(q, k, v, Wq, bq, Wk, bk, Wv, bv):
    in_maps = _marshal(q, k, v, Wq, bq, Wk, bk, Wv, bv)
    res = run_bass_kernel_spmd(_get_nc(), in_maps, core_ids=list(range(NCORES)))
    return _unmarshal(res.results)
